# revision 7
# baseline (speedup 1.0000x reference)
"""Trainium2 Bass kernel for nn_BaseKernelSetConv (gnn_message_passing).

v3 strategy (8 NeuronCores, data-parallel over focal nodes):

  The [128,1]-form indirect DMA costs ~1.4us per 128 gathered rows on the
  Pool engine (hard floor ~3.4ms/core for the 312k neighbor rows). v3
  replaces it with the custom SWDGE `dma_gather` ucode (InstDMAGatherAnt,
  transpose mode): ~1us fixed + ~0.34ns/row, up to 1024 rows/instruction,
  but int16 indices (32768-element windows) and 256B elements.

  To make every gather's index list dense and window-local:
    - per neighbor slot (d, j): focal columns are ORDERED BY SOURCE NODE
      (argsort of nei_index). Each 32768-element table chunk's entries
      then form a contiguous segment -> a few dma_gather instructions per
      (d,j,chunk) with local int16 indices, no holes.
    - the table is bf16 unit rows replicated 4x per 256B element;
      transpose mode emits gathered rows as COLUMNS (features on
      partitions 0..31) -> matmul rhs directly, no PE transposes.
    - per-slot score matrices [16, n] come back in slot-order; the HOST
      reorders and sums them into the final bands (free, mirrors the
      reference's argsort-reorder).
    - segment lengths are equalized across cores (SPMD shares one
      program); pads index element 0 of their chunk (valid rows, scores
      dropped on host), so index lists contain no -1 sentinels.
  Focal scores stay on the dense f32 path: host-permuted uxT stream ->
  one matmul per 512-column supertile.
"""

import sys
import numpy as np

sys.path.insert(0, "/opt/trn_rl_repo")

F = 32
K = 16
NCORES = 8
N = 1_000_000
SHARD = N // NCORES
GS = 4
C = 128 * GS              # focal supertile columns
CHUNK = 32768             # dma_gather int16 window (table elements)
NCHUNK = (N + CHUNK - 1) // CHUNK
MAXIDX = 512              # per-instruction num_idxs cap (desc ring limit;
                          # transpose-mode elements need 2x descriptors)
WINDOW = 4096             # matmul window columns
EB = 128                  # bf16 payload per table element (4x 32-bf16 row)

_PROG = None
NDC = {d: 0 for d in (1, 2, 3, 4)}
SEGS = None     # {(d,j): [(chunk, col_off, padded_len), ...]}  common layout
NDJ = None      # {(d,j): total padded columns}


def _slots():
    return [(d, j) for d in (1, 2, 3, 4) for j in range(d)]


def _freeze_layout(ndc, segs, ndj):
    global NDC, SEGS, NDJ, _PROG
    key = (tuple(sorted(ndc.items())),
           tuple((k, tuple(v)) for k, v in sorted(segs.items())),
           tuple(sorted(ndj.items())))
    cur = (tuple(sorted(NDC.items())),
           tuple((k, tuple(v)) for k, v in sorted((SEGS or {}).items())),
           tuple(sorted((NDJ or {}).items())))
    if key != cur:
        NDC = dict(ndc)
        SEGS = {k: list(v) for k, v in segs.items()}
        NDJ = dict(ndj)
        _PROG = None


def _freg():
    off, out = 0, {}
    for d in (1, 2, 3, 4):
        out[d] = off
        off += NDC[d]
    return out, off


def _nreg():
    off, out = 0, {}
    for d, j in _slots():
        out[(d, j)] = off
        off += NDJ[(d, j)]
    return out, off


def _pieces(d, j):
    """[(chunk, col_off, ni, idx_off)] in column order; idx_off is the
    piece's offset (in positions) into the flat idx tensor."""
    out = []
    for (ch, off, ln) in SEGS[(d, j)]:
        p = 0
        while p < ln:
            ni = min(MAXIDX, ln - p)
            out.append((ch, off + p, ni))
            p += ni
    return out


def _all_pieces():
    """{(d,j): [(chunk, col_off, ni, idx_off)]} with global idx offsets."""
    res = {}
    cur = 0
    for d, j in _slots():
        lst = []
        for (ch, coff, ni) in _pieces(d, j):
            lst.append((ch, coff, ni, cur))
            cur += ni
        res[(d, j)] = lst
    return res, cur


def _windows(pieces):
    """Group consecutive pieces into [wlo, whi) windows of <= WINDOW cols."""
    wins = []
    cur = []
    for pc in pieces:
        ch, coff, ni, ioff = pc
        if cur and (coff + ni - cur[0][1]) > WINDOW:
            wins.append((cur[0][1], cur[-1][1] + cur[-1][2], cur))
            cur = []
        cur.append(pc)
    if cur:
        wins.append((cur[0][1], cur[-1][1] + cur[-1][2], cur))
    return wins


def _build_program():
    import concourse.tile as tile
    from concourse import bacc, mybir

    f32 = mybir.dt.float32
    bf16 = mybir.dt.bfloat16
    i16 = mybir.dt.int16

    freg, FTOT = _freg()
    nreg, NTOT = _nreg()
    apieces, nidx = _all_pieces()
    assert nidx == NTOT, (nidx, NTOT)

    nc = bacc.Bacc("TRN2", target_bir_lowering=False, debug=False,
                   num_devices=NCORES)
    tab_d = nc.dram_tensor("tab", (NCHUNK * CHUNK, EB), bf16,
                           kind="ExternalInput").ap()
    uxt_d = nc.dram_tensor("uxt", (F, FTOT), f32, kind="ExternalInput").ap()
    wf_d = {d: nc.dram_tensor(f"wf{d}", (F, K), f32,
                              kind="ExternalInput").ap() for d in (1, 2, 3, 4)}
    wn_d = {(d, j): nc.dram_tensor(f"wn{d}_{j}", (F, K), bf16,
                                   kind="ExternalInput").ap()
            for d, j in _slots()}
    idx_d = nc.dram_tensor("idx", (8 * NTOT,), i16,
                           kind="ExternalInput").ap()
    outf_d = nc.dram_tensor("outf", (K, FTOT), f32,
                            kind="ExternalOutput").ap()
    outn_d = nc.dram_tensor("outn", (K, NTOT), f32,
                            kind="ExternalOutput").ap()

    with tile.TileContext(nc) as tc:
        with tc.tile_pool(name="wp", bufs=1) as wp, \
             tc.tile_pool(name="idxp", bufs=8) as idx_p, \
             tc.tile_pool(name="win", bufs=3) as win_p, \
             tc.tile_pool(name="uxt", bufs=3) as uxt_p, \
             tc.tile_pool(name="ost", bufs=4) as ost_p, \
             tc.tile_pool(name="fps", bufs=2, space="PSUM") as fps_p, \
             tc.tile_pool(name="nps", bufs=4, space="PSUM") as nps_p:

            wf_sb, wn_sb = {}, {}
            for d in (1, 2, 3, 4):
                wf_sb[d] = wp.tile([F, K], f32, tag=f"wf{d}", name=f"wf{d}")
                nc.sync.dma_start(wf_sb[d][:], wf_d[d][:])
            for d, j in _slots():
                wn_sb[(d, j)] = wp.tile([F, K], bf16, tag=f"wn{d}_{j}",
                                        name=f"wn{d}_{j}")
                nc.sync.dma_start(wn_sb[(d, j)][:], wn_d[(d, j)][:])

            def emit_focal(d, s):
                col = freg[d] + s * C
                ux = uxt_p.tile([F, C], f32, tag="ux", name="ux")
                nc.sync.dma_start(ux[:], uxt_d[:, col:col + C])
                ps = fps_p.tile([K, C], f32, tag="fps", name="fps")
                nc.tensor.matmul(ps[:], lhsT=wf_sb[d][:], rhs=ux[:],
                                 start=True, stop=True)
                ot = ost_p.tile([K, C], f32, tag="fot", name="fot")
                nc.scalar.copy(ot[:], ps[:])
                nc.sync.dma_start(outf_d[:, col:col + C], ot[:])

            copy_flip = [0]

            def emit_nei_window(d, j, wlo, whi, pieces):
                wn = whi - wlo
                win = win_p.tile([128, WINDOW], bf16, tag="win", name="win")
                for (ch, coff, ni, ioff) in pieces:
                    it = idx_p.tile([128, ni // 16], i16, tag="it", name="it")
                    nc.sync.dma_start(
                        it[:],
                        idx_d[8 * ioff:8 * (ioff + ni)]
                            .rearrange("(p c) -> p c", p=128))
                    nc.gpsimd.dma_gather(
                        out_ap=win[:, coff - wlo:coff - wlo + ni]
                            .rearrange("p (u n) -> p u n", u=1),
                        in_ap=tab_d[ch * CHUNK:(ch + 1) * CHUNK, :],
                        idxs_ap=it[:],
                        num_idxs=ni,
                        num_idxs_reg=ni,
                        elem_size=EB,
                        transpose=True,
                    )
                base = nreg[(d, j)]
                for a in range(0, wn, 512):
                    b = min(a + 512, wn)
                    ps = nps_p.tile([K, 512], f32, tag="nps", name="nps")
                    nc.tensor.matmul(ps[:, :b - a], lhsT=wn_sb[(d, j)][:],
                                     rhs=win[:F, a:b], start=True, stop=True)
                    ot = ost_p.tile([K, 512], f32, tag="not", name="not")
                    copy_flip[0] ^= 1
                    if copy_flip[0]:
                        nc.vector.tensor_copy(ot[:, :b - a], ps[:, :b - a])
                    else:
                        nc.scalar.copy(ot[:, :b - a], ps[:, :b - a])
                    nc.sync.dma_start(
                        outn_d[:, base + wlo + a:base + wlo + b],
                        ot[:, :b - a])

            nei_windows = []
            for d, j in _slots():
                for (wlo, whi, pcs) in _windows(apieces[(d, j)]):
                    nei_windows.append((d, j, wlo, whi, pcs))

            focal_items = [(d, s) for d in (1, 2, 3, 4)
                           for s in range(NDC[d] // C)]
            fi = 0
            nw = len(nei_windows)
            for i, w in enumerate(nei_windows):
                emit_nei_window(*w)
                while (fi < len(focal_items)
                       and fi + 1 <= (i + 1) * len(focal_items) // nw):
                    emit_focal(*focal_items[fi])
                    fi += 1
            while fi < len(focal_items):
                emit_focal(*focal_items[fi])
                fi += 1

    nc.compile()
    return nc


def _unit_rows(a):
    a = a.astype(np.float64)
    return (a / (np.linalg.norm(a, axis=-1, keepdims=True) + 1e-8)).astype(np.float32)


def _wrap16(ids):
    n = ids.shape[0]
    m = np.empty((16, n // 16), np.int16)
    m[np.arange(n) % 16, np.arange(n) // 16] = ids
    return np.tile(m, (8, 1)).reshape(-1)


def host_prep(inputs):
    import ml_dtypes
    bf16 = ml_dtypes.bfloat16

    x = np.asarray(inputs["x"], dtype=np.float32)
    sels = {d: np.asarray(inputs[f"selected_index_deg{d}"]).astype(np.int64)
            for d in (1, 2, 3, 4)}
    neis = {d: np.asarray(inputs[f"nei_index_deg{d}"]).astype(np.int64)
            .reshape(-1, d) for d in (1, 2, 3, 4)}

    ux = _unit_rows(x)

    deg = np.zeros(N, np.int8)
    pos = np.zeros(N, np.int64)
    for d in (1, 2, 3, 4):
        deg[sels[d]] = d
        pos[sels[d]] = np.arange(sels[d].shape[0])

    wf_lhsT = {}
    wn_lhsT = {}
    for d in (1, 2, 3, 4):
        wf_lhsT[d] = np.ascontiguousarray(
            _unit_rows(np.asarray(inputs[f"W_focal{d}"], np.float32)).T)
        wn = np.asarray(inputs[f"W_nei{d}"], np.float32)
        u = _unit_rows(wn.reshape(-1, F)).reshape(K, d, F) / d
        for j in range(d):
            wn_lhsT[(d, j)] = np.ascontiguousarray(u[:, j, :].T).astype(bf16)

    all_nodes = {}
    maxcnt = {d: 0 for d in (1, 2, 3, 4)}
    for c in range(NCORES):
        lo, hi = c * SHARD, (c + 1) * SHARD
        shard_deg = deg[lo:hi]
        for d in (1, 2, 3, 4):
            nodes_cd = np.nonzero(shard_deg == d)[0] + lo
            all_nodes[(c, d)] = nodes_cd
            maxcnt[d] = max(maxcnt[d], nodes_cd.shape[0])
    ndc = {d: ((maxcnt[d] + C - 1) // C) * C for d in (1, 2, 3, 4)}

    srcs = {}
    sorted_nodes = {}
    cnt_per_chunk = {}
    for c in range(NCORES):
        for d in (1, 2, 3, 4):
            nodes_cd = all_nodes[(c, d)]
            nei_cd = neis[d][pos[nodes_cd]]
            for j in range(d):
                sj = nei_cd[:, j]
                o = np.argsort(sj, kind="stable")
                srcs[(c, d, j)] = sj[o]
                sorted_nodes[(c, d, j)] = nodes_cd[o]
                cnt_per_chunk[(c, d, j)] = np.bincount(
                    sj[o] // CHUNK, minlength=NCHUNK)

    segs = {}
    ndj = {}
    for d, j in _slots():
        lens = np.zeros(NCHUNK, np.int64)
        for c in range(NCORES):
            lens = np.maximum(lens, cnt_per_chunk[(c, d, j)])
        lens = ((lens + 127) // 128) * 128
        seglist = []
        off = 0
        for ch in range(NCHUNK):
            seglist.append((ch, int(off), int(lens[ch])))
            off += int(lens[ch])
        segs[(d, j)] = seglist
        ndj[(d, j)] = int(off)

    _freeze_layout(ndc, segs, ndj)
    apieces, NTOT_idx = _all_pieces()
    freg, FTOT = _freg()
    nreg, NTOT = _nreg()
    assert NTOT_idx == NTOT

    uxb = ux.astype(bf16)
    tab = np.zeros((NCHUNK * CHUNK, EB), bf16)
    tab[:N] = np.tile(uxb, (1, 4))

    in_maps = []
    book = []
    for c in range(NCORES):
        m = {"tab": tab}
        bk = {"f": {}, "rp": {}, "sn": {}}
        uxt = np.zeros((FTOT, F), np.float32)
        for d in (1, 2, 3, 4):
            m[f"wf{d}"] = wf_lhsT[d]
            nodes_cd = all_nodes[(c, d)]
            cnt = nodes_cd.shape[0]
            uxt[freg[d]:freg[d] + cnt] = ux[nodes_cd]
            bk["f"][d] = (nodes_cd, cnt)
        m["uxt"] = np.ascontiguousarray(uxt.T)

        idx_flat = np.zeros(8 * NTOT, np.int16)
        for d, j in _slots():
            m[f"wn{d}_{j}"] = wn_lhsT[(d, j)]
            s_sorted = srcs[(c, d, j)]
            counts = cnt_per_chunk[(c, d, j)]
            local = (s_sorted % CHUNK).astype(np.int16)
            # real-entry positions in the common padded layout
            rp = np.empty(s_sorted.shape[0], np.int64)
            o = 0
            for (ch, off, ln) in SEGS[(d, j)]:
                k = int(counts[ch])
                rp[o:o + k] = off + np.arange(k)
                o += k
            bk["rp"][(d, j)] = rp
            bk["sn"][(d, j)] = sorted_nodes[(c, d, j)]
            # per-piece idx blocks (pads -> 0 = chunk's element 0)
            o = 0
            for (ch, off, ln) in SEGS[(d, j)]:
                if ln == 0:
                    continue
                k = int(counts[ch])
                chunk_ids = np.zeros(ln, np.int16)
                chunk_ids[:k] = local[o:o + k]
                o += k
            # walk pieces of this slot in order, consuming chunk_ids per seg
            # (redo loop aligned with piece enumeration)
            seg_ids = {}
            o = 0
            for (ch, off, ln) in SEGS[(d, j)]:
                k = int(counts[ch])
                ids = np.zeros(ln, np.int16)
                ids[:k] = local[o:o + k]
                o += k
                seg_ids[off] = ids
            for (ch, coff, ni, ioff) in apieces[(d, j)]:
                seg_off = None
                for (ch2, off2, ln2) in SEGS[(d, j)]:
                    if ch2 == ch and off2 <= coff < off2 + max(ln2, 1):
                        seg_off = off2
                        break
                ids = seg_ids[seg_off][coff - seg_off:coff - seg_off + ni]
                idx_flat[8 * ioff:8 * (ioff + ni)] = _wrap16(ids)
        m["idx"] = idx_flat
        in_maps.append(m)
        book.append(bk)
    return in_maps, book


def assemble(results, book):
    freg, _ = _freg()
    nreg, _ = _nreg()
    res = np.zeros((N, 4 * K), np.float32)
    for c in range(NCORES):
        outf = results[c]["outf"]
        outn = results[c]["outn"]
        for d in (1, 2, 3, 4):
            nodes_cd, cnt = book[c]["f"][d]
            band = slice(K * (d - 1), K * d)
            res[nodes_cd, band] = outf[:, freg[d]:freg[d] + cnt].T
        for d, j in _slots():
            rp = book[c]["rp"][(d, j)]
            nodes = book[c]["sn"][(d, j)]
            band = slice(K * (d - 1), K * d)
            res[nodes, band] += outn[:, nreg[(d, j)] + rp].T
    return res


LAST_RESULTS = None


def kernel(**inputs):
    global _PROG, LAST_RESULTS
    import os
    from concourse.bass_utils import run_bass_kernel_spmd
    in_maps, book = host_prep(inputs)
    if _PROG is None:
        _PROG = _build_program()
    trace = bool(os.environ.get("BKC_TRACE"))
    res = run_bass_kernel_spmd(_PROG, in_maps, core_ids=list(range(NCORES)),
                               trace=trace)
    LAST_RESULTS = res
    return assemble(res.results, book)


# ---------------------------------------------------------------------------
# numpy emulation of the device program (host-logic validation)
def _emulate_core(m, tab_f32):
    freg, FTOT = _freg()
    nreg, NTOT = _nreg()
    apieces, _ = _all_pieces()
    outf = np.zeros((K, FTOT), np.float32)
    uxt = m["uxt"].astype(np.float64)
    for d in (1, 2, 3, 4):
        sl = slice(freg[d], freg[d] + NDC[d])
        outf[:, sl] = (m[f"wf{d}"].astype(np.float64).T
                       @ uxt[:, sl]).astype(np.float32)
    outn = np.zeros((K, NTOT), np.float32)
    idx_flat = m["idx"]
    for d, j in _slots():
        w = m[f"wn{d}_{j}"].astype(np.float64)       # (F, K)
        for (ch, coff, ni, ioff) in apieces[(d, j)]:
            blk = idx_flat[8 * ioff:8 * (ioff + ni)].reshape(128, ni // 16)
            ids = np.empty(ni, np.int64)
            ids[:] = blk[np.arange(ni) % 16, np.arange(ni) // 16]
            rows = tab_f32[ch * CHUNK + ids]          # (ni, F) bf16->f32
            outn[:, nreg[(d, j)] + coff:nreg[(d, j)] + coff + ni] = \
                (w.T @ rows.T.astype(np.float64)).astype(np.float32)
    return {"outf": outf, "outn": outn}


def kernel_emulated(**inputs):
    in_maps, book = host_prep(inputs)
    tab_f32 = np.asarray(in_maps[0]["tab"][:, :F], dtype=np.float32)
    results = [_emulate_core(m, tab_f32) for m in in_maps]
    return assemble(results, book)


# revision 8
# speedup vs baseline: 1.1970x; 1.1970x over previous
"""Trainium2 Bass kernel for nn_BaseKernelSetConv (gnn_message_passing).

v2 strategy (8 NeuronCores, data-parallel over focal nodes):
  - Host pre-normalizes x (L2 rows) once -> the device does NO normalization.
  - Focal scores come from a DENSE stream: the host builds uxT_perm[32, L]
    whose columns are the core's focal nodes grouped by degree (ascending
    ids within a group, padded per supertile). One matmul per supertile
    against W_focal_d starts the PSUM accumulation.
  - Neighbor rows are gathered with [128,1]-form indirect DMAs (the only
    form the SWDGE ucode implements correctly; ~1.1us per 128 rows is the
    hard floor and the Pool engine must stay ~100% busy on exactly this).
    Gathered rows are PE-transposed into a [d*32, 512] rhs and one matmul
    accumulates all d neighbor slots into the same PSUM tile (W_nei rows
    pre-unit-normalized and pre-divided by d on host).
  - Out: fused [16, 512] band scores per supertile, assembled on host.
"""

import sys
import numpy as np

sys.path.insert(0, "/opt/trn_rl_repo")

F = 32
K = 16
NCORES = 8
N = 1_000_000
SHARD = N // NCORES
NPAD = 1_000_576          # padded gather-table rows
GS = 4                    # 128-col groups per supertile
C = 128 * GS              # focal columns per supertile (=512)

_PROG = None
NDC = {1: 25600, 2: 38400, 3: 38400, 4: 25600}   # padded per-(core,deg) counts


def _set_ndc(ndc):
    global NDC, _PROG
    if dict(ndc) != NDC:
        NDC = dict(ndc)
        _PROG = None


def _regions():
    """Per-degree column offsets in the fused output [16, LTOT]."""
    off, out = 0, {}
    for d in (1, 2, 3, 4):
        out[d] = off
        off += NDC[d]
    return out, off


def _build_program():
    import concourse.bass as bass
    import concourse.tile as tile
    from concourse import bacc, mybir
    from concourse.masks import make_identity

    f32 = mybir.dt.float32
    i32 = mybir.dt.int32

    reg, LTOT = _regions()

    nc = bacc.Bacc("TRN2", target_bir_lowering=False, debug=False,
                   num_devices=NCORES)
    x_d = nc.dram_tensor("x", (NPAD, F), f32, kind="ExternalInput").ap()
    uxt_d = nc.dram_tensor("uxt", (F, LTOT), f32, kind="ExternalInput").ap()
    wf_d = {d: nc.dram_tensor(f"wf{d}", (F, K), f32,
                              kind="ExternalInput").ap() for d in (1, 2, 3, 4)}
    wn_d = {d: nc.dram_tensor(f"wn{d}", (d * F, K), f32,
                              kind="ExternalInput").ap() for d in (1, 2, 3, 4)}
    idx_d = {d: nc.dram_tensor(f"idx{d}", (NDC[d] * d,), i32,
                               kind="ExternalInput").ap() for d in (1, 2, 3, 4)}
    out_o = nc.dram_tensor("out_o", (K, LTOT), f32,
                           kind="ExternalOutput").ap()

    with tile.TileContext(nc) as tc:
        with tc.tile_pool(name="wp", bufs=1) as wp, \
             tc.tile_pool(name="stage", bufs=4) as stage_p, \
             tc.tile_pool(name="uxt", bufs=3) as uxt_p, \
             tc.tile_pool(name="tsb", bufs=3) as tsb_p, \
             tc.tile_pool(name="ost", bufs=3) as ost_p, \
             tc.tile_pool(name="tps", bufs=3, space="PSUM") as tps_p, \
             tc.tile_pool(name="sps", bufs=2, space="PSUM") as sps_p:

            ident = wp.tile([128, 128], f32)
            make_identity(nc, ident[:])
            wf_sb, wn_sb, it_sb = {}, {}, {}
            for d in (1, 2, 3, 4):
                wf_sb[d] = wp.tile([F, K], f32, tag=f"wf{d}", name=f"wf{d}")
                nc.sync.dma_start(wf_sb[d][:], wf_d[d][:])
                wn_sb[d] = wp.tile([d * F, K], f32, tag=f"wn{d}",
                                   name=f"wn{d}")
                nc.sync.dma_start(wn_sb[d][:], wn_d[d][:])
                # whole idx table resident in SBUF: gathers never wait on
                # per-supertile idx loads
                cols = NDC[d] * d // 128
                it_sb[d] = wp.tile([128, cols], i32, tag=f"it{d}",
                                   name=f"it{d}")
                nc.sync.dma_start(
                    it_sb[d][:],
                    idx_d[d][:].rearrange("(p c) -> p c", p=128))

            def emit(d, s):
                """One supertile: C focals of degree d, positions
                [s*C, (s+1)*C) of the degree's region."""
                st = stage_p.tile([128, GS * d * F], f32, tag="stage",
                                  name="stage")
                base = s * GS * d
                for r in range(GS * d):
                    nc.gpsimd.indirect_dma_start(
                        out=st[:, r * F:(r + 1) * F],
                        out_offset=None,
                        in_=x_d[:],
                        in_offset=bass.IndirectOffsetOnAxis(
                            ap=it_sb[d][:, base + r:base + r + 1], axis=0),
                    )
                ts = tsb_p.tile([d * F, C], f32, tag="ts", name="ts")
                for g in range(GS):
                    tp = tps_p.tile([128, 128], f32, tag="tp", name="tp")
                    nc.tensor.transpose(
                        out=tp[:d * F, :],
                        in_=st[:, g * d * F:(g + 1) * d * F],
                        identity=ident[:])
                    nc.vector.tensor_copy(ts[:, g * 128:(g + 1) * 128],
                                          tp[:d * F, :])
                ux = uxt_p.tile([F, C], f32, tag="ux", name="ux")
                col = reg[d] + s * C
                nc.sync.dma_start(ux[:], uxt_d[:, col:col + C])
                ps = sps_p.tile([K, C], f32, tag="ps", name="ps")
                nc.tensor.matmul(ps[:], lhsT=wf_sb[d][:], rhs=ux[:],
                                 start=True, stop=False)
                nc.tensor.matmul(ps[:], lhsT=wn_sb[d][:], rhs=ts[:],
                                 start=False, stop=True)
                ot = ost_p.tile([K, C], f32, tag="ot", name="ot")
                nc.scalar.copy(ot[:], ps[:])
                nc.sync.dma_start(out_o[:, col:col + C], ot[:])

            items = [(d, s) for d in (1, 2, 3, 4)
                     for s in range(NDC[d] // C)]
            # round-robin across degrees to smooth PSUM/PE pressure
            items.sort(key=lambda t: (t[1], t[0]))
            for d, s in items:
                emit(d, s)

    nc.compile()
    return nc


def _unit_rows(a):
    a = a.astype(np.float64)
    return (a / (np.linalg.norm(a, axis=-1, keepdims=True) + 1e-8)).astype(np.float32)


def host_prep(inputs):
    x = np.asarray(inputs["x"], dtype=np.float32)
    sels = {d: np.asarray(inputs[f"selected_index_deg{d}"]).astype(np.int64)
            for d in (1, 2, 3, 4)}
    neis = {d: np.asarray(inputs[f"nei_index_deg{d}"]).astype(np.int64)
            .reshape(-1, d) for d in (1, 2, 3, 4)}

    # host-normalized gather table (pad rows benign)
    ux = _unit_rows(x)
    xpad = np.zeros((NPAD, F), np.float32)
    xpad[:N] = ux

    deg = np.zeros(N, np.int8)
    pos = np.zeros(N, np.int64)
    for d in (1, 2, 3, 4):
        deg[sels[d]] = d
        pos[sels[d]] = np.arange(sels[d].shape[0])

    wf_lhsT = {}
    wn_lhsT = {}
    for d in (1, 2, 3, 4):
        wf_lhsT[d] = np.ascontiguousarray(
            _unit_rows(np.asarray(inputs[f"W_focal{d}"], np.float32)).T)
        wn = np.asarray(inputs[f"W_nei{d}"], np.float32)
        u = _unit_rows(wn.reshape(-1, F)).reshape(K, d, F) / d
        wn_lhsT[d] = np.ascontiguousarray(u.reshape(K, d * F).T)

    all_nodes = {}
    maxcnt = {d: 0 for d in (1, 2, 3, 4)}
    for c in range(NCORES):
        lo, hi = c * SHARD, (c + 1) * SHARD
        shard_deg = deg[lo:hi]
        for d in (1, 2, 3, 4):
            nodes_cd = np.nonzero(shard_deg == d)[0] + lo
            all_nodes[(c, d)] = nodes_cd
            maxcnt[d] = max(maxcnt[d], nodes_cd.shape[0])
    _set_ndc({d: ((maxcnt[d] + C - 1) // C) * C for d in (1, 2, 3, 4)})
    reg, LTOT = _regions()

    in_maps = []
    book = []
    for c in range(NCORES):
        m = {"x": xpad}
        bk = {}
        uxt = np.zeros((LTOT, F), np.float32)
        for d in (1, 2, 3, 4):
            m[f"wf{d}"] = wf_lhsT[d]
            m[f"wn{d}"] = wn_lhsT[d]
            nodes_cd = all_nodes[(c, d)]
            cnt = nodes_cd.shape[0]
            uxt[reg[d]:reg[d] + cnt] = ux[nodes_cd]
            # neighbor ids per focal position, padded with 0
            nei_cd = np.zeros((NDC[d], d), np.int32)
            nei_cd[:cnt] = neis[d][pos[nodes_cd]].astype(np.int32)
            # device idx layout [128, NDC*d/128]: column (s*GS*d + g*d + j),
            # partition p  <->  focal position s*C + g*128 + p, slot j
            v = nei_cd.reshape(NDC[d] // C, GS, 128, d)     # s, g, p, j
            v = v.transpose(2, 0, 1, 3)                     # p, s, g, j
            m[f"idx{d}"] = np.ascontiguousarray(v).reshape(-1)
            bk[d] = (nodes_cd, cnt)
        m["uxt"] = np.ascontiguousarray(uxt.T)
        in_maps.append(m)
        book.append(bk)
    return in_maps, book


def assemble(results, book):
    reg, _ = _regions()
    res = np.zeros((N, 4 * K), np.float32)
    for c in range(NCORES):
        out = results[c]["out_o"]
        for d in (1, 2, 3, 4):
            nodes_cd, cnt = book[c][d]
            res[nodes_cd, K * (d - 1):K * d] = out[:, reg[d]:reg[d] + cnt].T
    return res


LAST_RESULTS = None


def kernel(**inputs):
    global _PROG, LAST_RESULTS
    import os
    from concourse.bass_utils import run_bass_kernel_spmd
    in_maps, book = host_prep(inputs)
    if _PROG is None:
        _PROG = _build_program()
    trace = bool(os.environ.get("BKC_TRACE"))
    res = run_bass_kernel_spmd(_PROG, in_maps, core_ids=list(range(NCORES)),
                               trace=trace)
    LAST_RESULTS = res
    return assemble(res.results, book)


# ---------------------------------------------------------------------------
# numpy emulation of the device program (host-logic validation)
def _emulate_core(m):
    reg, LTOT = _regions()
    x = m["x"].astype(np.float64)
    uxt = m["uxt"].astype(np.float64)
    out = np.zeros((K, LTOT), np.float32)
    for d in (1, 2, 3, 4):
        idx = m[f"idx{d}"].reshape(128, NDC[d] // C, GS, d)  # p, s, g, j
        idx = idx.transpose(1, 2, 0, 3).reshape(NDC[d], d)   # focal pos, j
        g = x[idx]                                           # (NDC, d, F)
        sc_n = np.einsum("ndf,dfk->kn", g,
                         m[f"wn{d}"].astype(np.float64).reshape(d, F, K))
        sc_f = m[f"wf{d}"].astype(np.float64).T @ uxt[:, reg[d]:reg[d] + NDC[d]]
        out[:, reg[d]:reg[d] + NDC[d]] = (sc_f + sc_n).astype(np.float32)
    return {"out_o": out}


def kernel_emulated(**inputs):
    in_maps, book = host_prep(inputs)
    results = [_emulate_core(m) for m in in_maps]
    return assemble(results, book)


# revision 17
# speedup vs baseline: 1.8724x; 1.5642x over previous
"""Trainium2 Bass kernel for nn_BaseKernelSetConv (gnn_message_passing).

v4 strategy (8 NeuronCores, data-parallel over focal nodes):

  Gathers run on the custom SWDGE `dma_gather` ucode (InstDMAGatherAnt,
  transpose mode) spread ROUND-ROBIN over 4 SWDGE queues: measured on HW,
  queue-rotated 512-idx gathers sustain ~5.1 ns/row vs ~11 ns/row for the
  engine-serial [128,1] indirect-DMA path (and vs ~9.2 ns/row for
  dma_gather on a single queue).

  dma_gather constraints shape the layout: int16 indices (32768-element
  windows), 256B elements, <=512 idxs/instruction (transpose descriptor
  ring). To keep every index list dense and window-local:
    - per neighbor slot (d, j): focal columns are ORDERED BY SOURCE NODE
      (argsort of nei_index); each 32768-element table chunk's entries
      form a contiguous segment -> a few dma_gathers per (d,j,chunk) with
      local int16 indices, no holes.
    - the table is bf16 unit rows replicated 4x per 256B element;
      transpose mode emits gathered rows as COLUMNS (features on
      partitions 0..31) -> matmul rhs directly, no PE transposes.
    - per-slot score matrices [16, n] come back in slot-order; the HOST
      reorders and sums them into the final bands (mirrors the
      reference's argsort-reorder step).
    - segment lengths are equalized across cores (SPMD shares one
      program); pads index element 0 of their chunk (valid rows, scores
      dropped on host) so index lists contain no -1 sentinels.
  Focal scores stay on the dense f32 path: host-permuted uxT stream ->
  one matmul per 512-column supertile. Host pre-normalizes x and weights.
"""

import sys
import numpy as np

sys.path.insert(0, "/opt/trn_rl_repo")

F = 32
K = 16
NCORES = 8
N = 1_000_000
SHARD = N // NCORES
GS = 4
C = 128 * GS              # focal supertile columns
CHUNK = 32768             # dma_gather int16 window (table elements)
NCHUNK = (N + CHUNK - 1) // CHUNK
MAXIDX = 512              # per-instruction num_idxs cap (transpose desc ring)
WINDOW = 4096             # matmul window columns
EB = 128                  # bf16 payload per table element (4x 32-bf16 row)
NQ = 4                    # SWDGE queues, round-robin

_PROG = None
NDC = {d: 0 for d in (1, 2, 3, 4)}
SEGS = None     # {(d,j): [(chunk, col_off, padded_len), ...]}  common layout
NDJ = None      # {(d,j): total padded columns}


def _slots():
    return [(d, j) for d in (1, 2, 3, 4) for j in range(d)]


def _freeze_layout(ndc, segs, ndj):
    global NDC, SEGS, NDJ, _PROG
    key = (tuple(sorted(ndc.items())),
           tuple((k, tuple(v)) for k, v in sorted(segs.items())),
           tuple(sorted(ndj.items())))
    cur = (tuple(sorted(NDC.items())),
           tuple((k, tuple(v)) for k, v in sorted((SEGS or {}).items())),
           tuple(sorted((NDJ or {}).items())))
    if key != cur:
        NDC = dict(ndc)
        SEGS = {k: list(v) for k, v in segs.items()}
        NDJ = dict(ndj)
        _PROG = None


def _freg():
    off, out = 0, {}
    for d in (1, 2, 3, 4):
        out[d] = off
        off += NDC[d]
    return out, off


def _nreg():
    off, out = 0, {}
    for d, j in _slots():
        out[(d, j)] = off
        off += NDJ[(d, j)]
    return out, off


def _pieces(d, j):
    out = []
    for (ch, off, ln) in SEGS[(d, j)]:
        p = 0
        while p < ln:
            ni = min(MAXIDX, ln - p)
            out.append((ch, off + p, ni))
            p += ni
    return out


def _all_pieces():
    """{(d,j): [(chunk, col_off, ni, idx_off)]} with global idx offsets."""
    res = {}
    cur = 0
    for d, j in _slots():
        lst = []
        for (ch, coff, ni) in _pieces(d, j):
            lst.append((ch, coff, ni, cur))
            cur += ni
        res[(d, j)] = lst
    return res, cur


def _windows(pieces):
    wins = []
    cur = []
    for pc in pieces:
        ch, coff, ni, ioff = pc
        if cur and (coff + ni - cur[0][1]) > WINDOW:
            wins.append((cur[0][1], cur[-1][1] + cur[-1][2], cur))
            cur = []
        cur.append(pc)
    if cur:
        wins.append((cur[0][1], cur[-1][1] + cur[-1][2], cur))
    return wins


def _build_program():
    import concourse.tile as tile
    from concourse import bacc, mybir

    f32 = mybir.dt.float32
    bf16 = mybir.dt.bfloat16
    i16 = mybir.dt.int16

    freg, FTOT = _freg()
    nreg, NTOT = _nreg()
    apieces, nidx = _all_pieces()
    assert nidx == NTOT, (nidx, NTOT)

    nc = bacc.Bacc("TRN2", target_bir_lowering=False, debug=False,
                   num_devices=NCORES, num_swdge_queues=NQ)
    tab_d = nc.dram_tensor("tab", (NCHUNK * CHUNK, EB), bf16,
                           kind="ExternalInput").ap()
    uxt_d = nc.dram_tensor("uxt", (F, FTOT), f32, kind="ExternalInput").ap()
    wf_d = {d: nc.dram_tensor(f"wf{d}", (F, K), f32,
                              kind="ExternalInput").ap() for d in (1, 2, 3, 4)}
    wn_d = {(d, j): nc.dram_tensor(f"wn{d}_{j}", (F, K), bf16,
                                   kind="ExternalInput").ap()
            for d, j in _slots()}
    idx_d = nc.dram_tensor("idx", (8 * NTOT,), i16,
                           kind="ExternalInput").ap()
    outf_d = nc.dram_tensor("outf", (K, FTOT), f32,
                            kind="ExternalOutput").ap()
    outn_d = nc.dram_tensor("outn", (K, NTOT), f32,
                            kind="ExternalOutput").ap()

    with tile.TileContext(nc) as tc:
        with tc.tile_pool(name="wp", bufs=1) as wp, \
             tc.tile_pool(name="idxp", bufs=8) as idx_p, \
             tc.tile_pool(name="win", bufs=3) as win_p, \
             tc.tile_pool(name="uxt", bufs=3) as uxt_p, \
             tc.tile_pool(name="ost", bufs=4) as ost_p, \
             tc.tile_pool(name="fps", bufs=2, space="PSUM") as fps_p, \
             tc.tile_pool(name="nps", bufs=4, space="PSUM") as nps_p:

            wf_sb, wn_sb = {}, {}
            for d in (1, 2, 3, 4):
                wf_sb[d] = wp.tile([F, K], f32, tag=f"wf{d}", name=f"wf{d}")
                nc.sync.dma_start(wf_sb[d][:], wf_d[d][:])
            for d, j in _slots():
                wn_sb[(d, j)] = wp.tile([F, K], bf16, tag=f"wn{d}_{j}",
                                        name=f"wn{d}_{j}")
                nc.sync.dma_start(wn_sb[(d, j)][:], wn_d[(d, j)][:])

            def emit_focal(d, s):
                col = freg[d] + s * C
                ux = uxt_p.tile([F, C], f32, tag="ux", name="ux")
                nc.sync.dma_start(ux[:], uxt_d[:, col:col + C])
                ps = fps_p.tile([K, C], f32, tag="fps", name="fps")
                nc.tensor.matmul(ps[:], lhsT=wf_sb[d][:], rhs=ux[:],
                                 start=True, stop=True)
                ot = ost_p.tile([K, C], f32, tag="fot", name="fot")
                nc.scalar.copy(ot[:], ps[:])
                nc.sync.dma_start(outf_d[:, col:col + C], ot[:])

            qctr = [0]
            copy_flip = [0]

            def emit_nei_window(d, j, wlo, whi, pieces):
                wn = whi - wlo
                win = win_p.tile([128, WINDOW], bf16, tag="win", name="win")
                for (ch, coff, ni, ioff) in pieces:
                    it = idx_p.tile([128, ni // 16], i16, tag="it", name="it")
                    nc.sync.dma_start(
                        it[:],
                        idx_d[8 * ioff:8 * (ioff + ni)]
                            .rearrange("(p c) -> p c", p=128))
                    nc.gpsimd.dma_gather(
                        out_ap=win[:, coff - wlo:coff - wlo + ni]
                            .rearrange("p (u n) -> p u n", u=1),
                        in_ap=tab_d[ch * CHUNK:(ch + 1) * CHUNK, :],
                        idxs_ap=it[:],
                        num_idxs=ni,
                        num_idxs_reg=ni,
                        elem_size=EB,
                        transpose=True,
                        queue_num=qctr[0] % NQ,
                    )
                    qctr[0] += 1
                base = nreg[(d, j)]
                for a in range(0, wn, 512):
                    b = min(a + 512, wn)
                    ps = nps_p.tile([K, 512], f32, tag="nps", name="nps")
                    nc.tensor.matmul(ps[:, :b - a], lhsT=wn_sb[(d, j)][:],
                                     rhs=win[:F, a:b], start=True, stop=True)
                    ot = ost_p.tile([K, 512], f32, tag="not", name="not")
                    copy_flip[0] ^= 1
                    if copy_flip[0]:
                        nc.vector.tensor_copy(ot[:, :b - a], ps[:, :b - a])
                    else:
                        nc.scalar.copy(ot[:, :b - a], ps[:, :b - a])
                    nc.sync.dma_start(
                        outn_d[:, base + wlo + a:base + wlo + b],
                        ot[:, :b - a])

            nei_windows = []
            for d, j in _slots():
                for (wlo, whi, pcs) in _windows(apieces[(d, j)]):
                    nei_windows.append((d, j, wlo, whi, pcs))

            focal_items = [(d, s) for d in (1, 2, 3, 4)
                           for s in range(NDC[d] // C)]
            fi = 0
            nw = len(nei_windows)
            for i, w in enumerate(nei_windows):
                emit_nei_window(*w)
                while (fi < len(focal_items)
                       and fi + 1 <= (i + 1) * len(focal_items) // nw):
                    emit_focal(*focal_items[fi])
                    fi += 1
            while fi < len(focal_items):
                emit_focal(*focal_items[fi])
                fi += 1

    nc.compile()
    return nc


def _unit_rows(a):
    a = a.astype(np.float64)
    return (a / (np.linalg.norm(a, axis=-1, keepdims=True) + 1e-8)).astype(np.float32)


def _wrap16(ids):
    n = ids.shape[0]
    m = np.empty((16, n // 16), np.int16)
    m[np.arange(n) % 16, np.arange(n) // 16] = ids
    return np.tile(m, (8, 1)).reshape(-1)


def host_prep(inputs):
    import ml_dtypes
    bf16 = ml_dtypes.bfloat16

    x = np.asarray(inputs["x"], dtype=np.float32)
    sels = {d: np.asarray(inputs[f"selected_index_deg{d}"]).astype(np.int64)
            for d in (1, 2, 3, 4)}
    neis = {d: np.asarray(inputs[f"nei_index_deg{d}"]).astype(np.int64)
            .reshape(-1, d) for d in (1, 2, 3, 4)}

    ux = _unit_rows(x)

    deg = np.zeros(N, np.int8)
    pos = np.zeros(N, np.int64)
    for d in (1, 2, 3, 4):
        deg[sels[d]] = d
        pos[sels[d]] = np.arange(sels[d].shape[0])

    wf_lhsT = {}
    wn_lhsT = {}
    for d in (1, 2, 3, 4):
        wf_lhsT[d] = np.ascontiguousarray(
            _unit_rows(np.asarray(inputs[f"W_focal{d}"], np.float32)).T)
        wn = np.asarray(inputs[f"W_nei{d}"], np.float32)
        u = _unit_rows(wn.reshape(-1, F)).reshape(K, d, F) / d
        for j in range(d):
            wn_lhsT[(d, j)] = np.ascontiguousarray(u[:, j, :].T).astype(bf16)

    all_nodes = {}
    maxcnt = {d: 0 for d in (1, 2, 3, 4)}
    for c in range(NCORES):
        lo, hi = c * SHARD, (c + 1) * SHARD
        shard_deg = deg[lo:hi]
        for d in (1, 2, 3, 4):
            nodes_cd = np.nonzero(shard_deg == d)[0] + lo
            all_nodes[(c, d)] = nodes_cd
            maxcnt[d] = max(maxcnt[d], nodes_cd.shape[0])
    ndc = {d: ((maxcnt[d] + C - 1) // C) * C for d in (1, 2, 3, 4)}

    srcs = {}
    sorted_nodes = {}
    cnt_per_chunk = {}
    for c in range(NCORES):
        for d in (1, 2, 3, 4):
            nodes_cd = all_nodes[(c, d)]
            nei_cd = neis[d][pos[nodes_cd]]
            for j in range(d):
                sj = nei_cd[:, j]
                o = np.argsort(sj, kind="stable")
                srcs[(c, d, j)] = sj[o]
                sorted_nodes[(c, d, j)] = nodes_cd[o]
                cnt_per_chunk[(c, d, j)] = np.bincount(
                    sj[o] // CHUNK, minlength=NCHUNK)

    segs = {}
    ndj = {}
    for d, j in _slots():
        lens = np.zeros(NCHUNK, np.int64)
        for c in range(NCORES):
            lens = np.maximum(lens, cnt_per_chunk[(c, d, j)])
        lens = ((lens + 127) // 128) * 128
        seglist = []
        off = 0
        for ch in range(NCHUNK):
            seglist.append((ch, int(off), int(lens[ch])))
            off += int(lens[ch])
        segs[(d, j)] = seglist
        ndj[(d, j)] = int(off)

    _freeze_layout(ndc, segs, ndj)
    apieces, NTOT_idx = _all_pieces()
    freg, FTOT = _freg()
    nreg, NTOT = _nreg()
    assert NTOT_idx == NTOT

    uxb = ux.astype(bf16)
    tab = np.zeros((NCHUNK * CHUNK, EB), bf16)
    tab[:N] = np.tile(uxb, (1, 4))

    in_maps = []
    book = []
    for c in range(NCORES):
        m = {"tab": tab}
        bk = {"f": {}, "rp": {}, "sn": {}}
        uxt = np.zeros((FTOT, F), np.float32)
        for d in (1, 2, 3, 4):
            m[f"wf{d}"] = wf_lhsT[d]
            nodes_cd = all_nodes[(c, d)]
            cnt = nodes_cd.shape[0]
            uxt[freg[d]:freg[d] + cnt] = ux[nodes_cd]
            bk["f"][d] = (nodes_cd, cnt)
        m["uxt"] = np.ascontiguousarray(uxt.T)

        idx_flat = np.zeros(8 * NTOT, np.int16)
        for d, j in _slots():
            m[f"wn{d}_{j}"] = wn_lhsT[(d, j)]
            s_sorted = srcs[(c, d, j)]
            counts = cnt_per_chunk[(c, d, j)]
            local = (s_sorted % CHUNK).astype(np.int16)
            rp = np.empty(s_sorted.shape[0], np.int64)
            o = 0
            for (ch, off, ln) in SEGS[(d, j)]:
                k = int(counts[ch])
                rp[o:o + k] = off + np.arange(k)
                o += k
            bk["rp"][(d, j)] = rp
            bk["sn"][(d, j)] = sorted_nodes[(c, d, j)]
            seg_ids = {}
            o = 0
            for (ch, off, ln) in SEGS[(d, j)]:
                k = int(counts[ch])
                ids = np.zeros(ln, np.int16)
                ids[:k] = local[o:o + k]
                o += k
                seg_ids[off] = ids
            seg_offs = {off: (ch, ln) for (ch, off, ln) in SEGS[(d, j)]}
            seg_starts = sorted(seg_offs.keys())
            import bisect
            for (ch, coff, ni, ioff) in apieces[(d, j)]:
                si = seg_starts[bisect.bisect_right(seg_starts, coff) - 1]
                ids = seg_ids[si][coff - si:coff - si + ni]
                idx_flat[8 * ioff:8 * (ioff + ni)] = _wrap16(ids)
        m["idx"] = idx_flat
        in_maps.append(m)
        book.append(bk)
    return in_maps, book


def assemble(results, book):
    freg, _ = _freg()
    nreg, _ = _nreg()
    res = np.zeros((N, 4 * K), np.float32)
    for c in range(NCORES):
        outf = results[c]["outf"]
        outn = results[c]["outn"]
        for d in (1, 2, 3, 4):
            nodes_cd, cnt = book[c]["f"][d]
            band = slice(K * (d - 1), K * d)
            res[nodes_cd, band] = outf[:, freg[d]:freg[d] + cnt].T
        for d, j in _slots():
            rp = book[c]["rp"][(d, j)]
            nodes = book[c]["sn"][(d, j)]
            band = slice(K * (d - 1), K * d)
            res[nodes, band] += outn[:, nreg[(d, j)] + rp].T
    return res


LAST_RESULTS = None


def kernel(**inputs):
    global _PROG, LAST_RESULTS
    import os
    from concourse.bass_utils import run_bass_kernel_spmd
    in_maps, book = host_prep(inputs)
    if _PROG is None:
        _PROG = _build_program()
    trace = bool(os.environ.get("BKC_TRACE"))
    res = run_bass_kernel_spmd(_PROG, in_maps, core_ids=list(range(NCORES)),
                               trace=trace)
    LAST_RESULTS = res
    return assemble(res.results, book)


# ---------------------------------------------------------------------------
# numpy emulation of the device program (host-logic validation)
def _emulate_core(m, tab_f32):
    freg, FTOT = _freg()
    nreg, NTOT = _nreg()
    apieces, _ = _all_pieces()
    outf = np.zeros((K, FTOT), np.float32)
    uxt = m["uxt"].astype(np.float64)
    for d in (1, 2, 3, 4):
        sl = slice(freg[d], freg[d] + NDC[d])
        outf[:, sl] = (m[f"wf{d}"].astype(np.float64).T
                       @ uxt[:, sl]).astype(np.float32)
    outn = np.zeros((K, NTOT), np.float32)
    idx_flat = m["idx"]
    for d, j in _slots():
        w = m[f"wn{d}_{j}"].astype(np.float64)
        for (ch, coff, ni, ioff) in apieces[(d, j)]:
            blk = idx_flat[8 * ioff:8 * (ioff + ni)].reshape(128, ni // 16)
            ids = np.empty(ni, np.int64)
            ids[:] = blk[np.arange(ni) % 16, np.arange(ni) // 16]
            rows = tab_f32[ch * CHUNK + ids]
            outn[:, nreg[(d, j)] + coff:nreg[(d, j)] + coff + ni] = \
                (w.T @ rows.T.astype(np.float64)).astype(np.float32)
    return {"outf": outf, "outn": outn}


def kernel_emulated(**inputs):
    in_maps, book = host_prep(inputs)
    tab_f32 = np.asarray(in_maps[0]["tab"][:, :F], dtype=np.float32)
    results = [_emulate_core(m) if False else _emulate_core(m, tab_f32)
               for m in in_maps]
    return assemble(results, book)
